# revision 1
# baseline (speedup 1.0000x reference)
"""Causal ReLU-attention block (qkv proj + per-head attention) on 8 trn2 cores.

Sharding: pure data-parallel over batch (B=8 -> 1 batch element per core).
Per-core: x_b [T,C] -> qkv -> scoresT = k q^T (row-tiled head pairs) ->
relu/causal-mask -> yT = v.T @ scoresT (col-tiled head pairs) -> DRAM yT [C,T].
Host side: transpose/cast/regroup shards in, transpose gather out.

Schedule: W re-laid out on host into priority blocks so all input staging
is a handful of coarse DMAs on the sync queue (per-transfer issue costs
~0.6us of the issuing engine's sequencer); PE warmup matmuls (a prefix plus
bursts between early work units) keep the HAM clock-gate warm through the
DMA-bound ramp; qk/v emission for later head pairs is injected into the
attention groups so the PSUM-eviction work (ACT/DVE) spreads across the
whole kernel; evictions are greedily balanced between ACT and DVE with a
local-in-time decay; bf16 output, per-q-chunk stores, last group's streams
store on different DMA queues so the final receipts overlap.
"""

import sys
from contextlib import ExitStack

sys.path.insert(0, "/opt/trn_rl_repo")

import ml_dtypes
import numpy as np

import concourse.bass as bass
import concourse.tile as tile
from concourse import bacc, bass_utils, mybir

P = 128
QW = 512  # t_q chunk width (PSUM bank = 512 fp32)
T, C, H = 1024, 768, 12
CT = C // P
TT = T // P
NQC = T // QW
VW = C // 2
NHP = H // 2
HD = C // H

# ---- host-side W staging layout (free-dim offsets, in elements) -----------
# block A0: qk tiles for head pair 0     -> per ct: [q0 k0] * 128
# block A1: qk tiles for head pair 1     -> per ct: [q1 k1] * 128
# block B:  v weights                    -> per ct: [oc0 oc1] * 384
# block C:  qk tiles for head pairs 2..5 -> per ct: [q2..q5 k2..k5] * 128
A0_BASE = 0
A1_BASE = 6 * 256
B_BASE, B_CT = A1_BASE + 6 * 256, 768
C_BASE, C_CT = B_BASE + 6 * 768, 1024
W_TOTAL = C_BASE + 6 * 1024


def qk_off(ct, ot):
    """Offset of the 128-wide qk weight tile (ot: 0..5 = q, 6..11 = k)."""
    q = ot < CT
    pair = (ot if q else ot - CT)
    if pair < 2:
        base = A0_BASE if pair == 0 else A1_BASE
        return base + ct * 256 + (0 if q else P)
    j = (pair - 2) + (0 if q else 4)
    return C_BASE + ct * C_CT + j * P


def v_off(ct, oc):
    return B_BASE + ct * B_CT + oc * VW


BF16 = mybir.dt.bfloat16
F32 = mybir.dt.float32
AF = mybir.ActivationFunctionType
ALU = mybir.AluOpType


def build_module(n_cores=8):
    """Build + compile the per-core Bass module (same program on all cores)."""
    scale = 1.0 / float(np.sqrt(HD))

    nc = bacc.Bacc("TRN2", target_bir_lowering=False, debug=False,
                   num_devices=n_cores)

    xT = nc.dram_tensor("xT", [C, T], BF16, kind="ExternalInput").ap()
    wS = nc.dram_tensor("wS", [P, W_TOTAL], BF16, kind="ExternalInput").ap()
    bqk = nc.dram_tensor("bqk", [P, 2 * CT], F32, kind="ExternalInput").ap()
    bv = nc.dram_tensor("bv", [P, C], BF16, kind="ExternalInput").ap()
    yT = nc.dram_tensor("yT", [C, T], BF16, kind="ExternalOutput").ap()

    xT3 = xT.rearrange("(ct p) t -> p ct t", p=P)

    with tile.TileContext(nc) as tc, ExitStack() as ctx:
        const = ctx.enter_context(tc.tile_pool(name="const", bufs=1))
        psum = ctx.enter_context(tc.tile_pool(name="psum", bufs=3, space="PSUM"))
        ypsum = ctx.enter_context(tc.tile_pool(name="ypsum", bufs=2, space="PSUM"))
        scb = ctx.enter_context(tc.tile_pool(name="scb", bufs=14))
        ysb = ctx.enter_context(tc.tile_pool(name="ysb", bufs=4))

        wt_sb = const.tile([P, W_TOTAL], BF16)
        xt_sb = const.tile([P, CT, T], BF16)
        bqk_sb = const.tile([P, 2 * CT], F32)
        bv_sb = const.tile([P, 2, VW], BF16)

        # coarse, priority-ordered staging; everything on the (otherwise
        # idle) sync queue so no compute engine pays DMA-issue time
        nc.sync.dma_start(bqk_sb[:], bqk[:])
        # first-needed blocks split in half: the qk contraction's per-ct
        # matmuls gate on subtile deps, so the first cts can start while the
        # second halves are still in flight
        half = (A1_BASE - A0_BASE) // 2
        nc.sync.dma_start(wt_sb[:, A0_BASE:A0_BASE + half],
                          wS[:, A0_BASE:A0_BASE + half])
        nc.sync.dma_start(xt_sb[:, 0:CT // 2, 0:QW], xT3[:, 0:CT // 2, 0:QW])
        nc.sync.dma_start(wt_sb[:, A0_BASE + half:A1_BASE],
                          wS[:, A0_BASE + half:A1_BASE])
        nc.sync.dma_start(xt_sb[:, CT // 2:CT, 0:QW],
                          xT3[:, CT // 2:CT, 0:QW])
        nc.sync.dma_start(wt_sb[:, A1_BASE:B_BASE], wS[:, A1_BASE:B_BASE])
        nc.sync.dma_start(xt_sb[:, :, QW:T], xT3[:, :, QW:T])
        nc.sync.dma_start(wt_sb[:, B_BASE:C_BASE], wS[:, B_BASE:C_BASE])
        nc.sync.dma_start(bv_sb[:], bv.rearrange("p (oc v) -> p oc v", oc=2))
        nc.sync.dma_start(wt_sb[:, C_BASE:W_TOTAL], wS[:, C_BASE:W_TOTAL])

        # PE warmup: keep the HAM activity window busy while staging lands.
        # A prefix covers the fixed preamble+first-transfer latency; small
        # bursts interleaved between the early work units then absorb the
        # remaining DMA stalls without delaying ready work by much.
        warm_sb = const.tile([P, P], BF16)
        nc.gpsimd.memset(warm_sb[:], 0.0)
        # lives in the ypsum ring: every burst precedes the first y tile, so
        # the slot rotates away cleanly before attention needs it
        warm_ps = ypsum.tile([P, QW], F32, tag="y", name="warm")

        def warm_burst(n):
            for _ in range(n):
                nc.tensor.matmul(warm_ps[:, 0:64], warm_sb[:],
                                 warm_sb[:, 0:64], start=True, stop=True)

        warm_burst(36)

        qkT = const.tile([P, 2 * CT, T], BF16)   # o-tiles: q = 0..CT-1, k = CT..
        vsb = const.tile([P, TT, C], BF16)       # v in natural [t, o] layout

        # ---- eviction engine balancing ------------------------------------
        load = {"act": 0.0, "dve": 0.0}

        def pick_engine(n_elem):
            # decay so the balance is local in time, not just global --
            # otherwise early asymmetries (qk->ACT, v->DVE) leave one engine
            # saturated in the injection-free end phase
            load["act"] *= 0.95
            load["dve"] *= 0.95
            ca = (n_elem + 352) / 1.2
            cd = n_elem / 0.96 + 160.0
            if load["act"] + ca <= load["dve"] + cd:
                load["act"] += ca
                return "act"
            load["dve"] += cd
            return "dve"

        def relu_evict(dst, src, n_elem):
            if pick_engine(n_elem) == "act":
                nc.scalar.activation(dst, src, AF.Relu, scale=scale)
            else:
                nc.vector.tensor_scalar(dst, src, scale, 0.0, ALU.mult, ALU.max)

        def copy_evict(dst, src, n_elem):
            if pick_engine(n_elem) == "act":
                nc.scalar.activation(dst, src, AF.Copy)
            else:
                nc.vector.tensor_copy(dst, src)

        # ---- qkv projection pieces ----------------------------------------
        proj_state = {}

        def emit_qk_half(ot, qc, split_evict=False):
            if ot not in proj_state:
                proj_state[ot] = psum.tile([P, 2, QW], F32, tag="blk",
                                           name="qk_ps")
            ps = proj_state[ot]
            for ct in range(CT):
                off = qk_off(ct, ot)
                nc.tensor.matmul(
                    ps[:, qc],
                    wt_sb[:, off:off + P],
                    xt_sb[:, ct, qc * QW:(qc + 1) * QW],
                    start=(ct == 0), stop=(ct == CT - 1),
                )
            if split_evict:
                # evict this q-chunk as soon as its chain completes: its
                # consumers unblock ~a chunk earlier, and the extra per-
                # instruction overhead lands where ACT/DVE are idle anyway
                qk3 = qkT[:, ot].rearrange("p (a b) -> p a b", a=NQC)
                if pick_engine(QW) == "act":
                    nc.scalar.activation(
                        qk3[:, qc], ps[:, qc],
                        AF.Identity, bias=bqk_sb[:, ot:ot + 1])
                else:
                    nc.vector.tensor_tensor(
                        qk3[:, qc:qc + 1], ps[:, qc:qc + 1],
                        bqk_sb[:, ot:ot + 1, None].to_broadcast((P, 1, QW)),
                        ALU.add)
                if qc == NQC - 1:
                    proj_state.pop(ot)
                return
            if qc == NQC - 1:
                ps = proj_state.pop(ot)
                n = NQC * QW
                if pick_engine(n) == "act":
                    nc.scalar.activation(
                        qkT[:, ot], ps[:, :NQC].rearrange("p a b -> p (a b)"),
                        AF.Identity, bias=bqk_sb[:, ot:ot + 1])
                else:
                    nc.vector.tensor_tensor(
                        qkT[:, ot].rearrange("p (a b) -> p a b", a=NQC),
                        ps[:, :NQC],
                        bqk_sb[:, ot:ot + 1, None].to_broadcast((P, NQC, QW)),
                        ALU.add)

        def emit_qk(ot):
            for qc in range(NQC):
                emit_qk_half(ot, qc)

        def emit_v_half(tt, oc):
            if ("v", tt) not in proj_state:
                proj_state[("v", tt)] = psum.tile([P, 2, QW], F32, tag="blk",
                                                  name="v_ps")
            ps = proj_state[("v", tt)]
            for ct in range(CT):
                off = v_off(ct, oc)
                nc.tensor.matmul(
                    ps[:, oc, :VW],
                    xt_sb[:, ct, tt * P:(tt + 1) * P],
                    wt_sb[:, off:off + VW],
                    start=(ct == 0), stop=(ct == CT - 1),
                )
            if oc == 1:
                ps = proj_state.pop(("v", tt))
                # bias varies along the free dim -> DVE tensor_tensor only
                load["dve"] += C / 0.96 + 160.0
                nc.vector.tensor_tensor(
                    vsb[:, tt].rearrange("p (oc v) -> p oc v", oc=2),
                    ps[:, :, :VW], bv_sb[:], ALU.add)

        # ---- attention -----------------------------------------------------
        def attention_closures(hp, store_eng=None):
            items = []
            for qc in range(NQC):
                kb_hi = min((qc * QW + QW - 1) // P, TT - 1)
                for kb in range(kb_hi + 1):
                    items.append((qc, kb, kb_hi))
            state = {"s": {}, "y": {}}
            sc_fns, av_fns = [], []

            def sc(i, qc, kb, kb_hi):
                delta = max(kb * P - qc * QW, 0)   # first valid t_q col
                sp = psum.tile([P, 2, QW], F32, tag="blk", name="s_ps")
                for h, ppos in ((0, (0, 0)), (1, (64, 0))):
                    nc.tensor.matmul(
                        sp[:, h, delta:QW],
                        qkT[h * 64:(h + 1) * 64, CT + hp,
                            kb * P:(kb + 1) * P],
                        qkT[h * 64:(h + 1) * 64, hp,
                            qc * QW + delta:(qc + 1) * QW],
                        start=True, stop=True, tile_position=ppos,
                    )
                s = scb.tile([P, 2, QW], BF16, tag="s")
                relu_evict(s[:, :, delta:QW], sp[:, :, delta:QW],
                           2 * (QW - delta))
                if kb * P >= qc * QW:   # diagonal block: causal mask on the
                    # first P cols only (row p can only mask j' < p < P)
                    nc.gpsimd.affine_select(
                        s[:, :, delta:delta + P],
                        s[:, :, delta:delta + P],
                        pattern=[[0, 2], [1, P]],
                        compare_op=ALU.is_ge, fill=0.0,
                        base=0, channel_multiplier=-1,
                    )
                state["s"][i] = s

            def av(i, qc, kb, kb_hi):
                if kb == 0:
                    state["y"][qc] = ypsum.tile([P, QW], F32, tag="y",
                                                name="yp")
                yp = state["y"][qc]
                delta = max(kb * P - qc * QW, 0)
                s = state["s"].pop(i)
                # two heads accumulate into disjoint partition ranges of one
                # bank; each runs its own start/stop group (the sim's group
                # checker can't see base partition -> skip)
                nc.tensor.matmul(
                    yp[0:64, delta:QW], vsb[:, kb, hp * P:hp * P + 64],
                    s[:, 0, delta:QW],
                    start=(kb == 0), stop=(kb == kb_hi),
                    tile_position=(0, 0), skip_group_check=True,
                )
                nc.tensor.matmul(
                    yp[64:128, delta:QW],
                    vsb[:, kb, hp * P + 64:hp * P + 128],
                    s[:, 1, delta:QW],
                    start=(kb == 0), stop=(kb == kb_hi),
                    tile_position=(0, 64), skip_group_check=True,
                )
                if kb == kb_hi:
                    yp = state["y"].pop(qc)
                    yt = ysb.tile([P, QW], BF16, tag="yt")
                    copy_evict(yt[:], yp[:], QW)
                    eng = store_eng if store_eng is not None else nc.sync
                    eng.dma_start(
                        yT[hp * P:(hp + 1) * P, qc * QW:(qc + 1) * QW],
                        yt[:])

            for i, (qc, kb, kb_hi) in enumerate(items):
                sc_fns.append(
                    lambda i=i, qc=qc, kb=kb, kb_hi=kb_hi: sc(i, qc, kb, kb_hi))
                av_fns.append(
                    lambda i=i, qc=qc, kb=kb, kb_hi=kb_hi: av(i, qc, kb, kb_hi))
            return sc_fns, av_fns

        def run_group(hps, injections, last=False, stagger=0, span_steps=None):
            # on the last group, put the two streams' stores on different
            # queues so the final completion receipts overlap; `stagger`
            # delays the second stream so injections can feed its inputs
            streams = [attention_closures(
                hp, nc.scalar if (last and j == 1) else None)
                for j, hp in enumerate(hps)]
            LAG = 3
            n = len(streams[0][0])
            total = n + LAG + stagger * (len(streams) - 1)
            m = len(injections)
            done = 0
            # front-load injections (finish by ~3/4 through the group) so the
            # next group's qkT/v dependencies are evicted before it starts
            span = span_steps if span_steps else max(1, (3 * total) // 4)
            for i in range(total):
                for j, (sc_fns, _) in enumerate(streams):
                    ii = i - stagger * j
                    if 0 <= ii < n:
                        sc_fns[ii]()
                due = min(m, (i + 1) * m // span)
                while done < due:
                    injections[done]()
                    done += 1
                for j, (_, av_fns) in enumerate(streams):
                    ii = i - LAG - stagger * j
                    if 0 <= ii < n:
                        av_fns[ii]()

        # ---- schedule ------------------------------------------------------
        # q,k for pairs 0,1; qc0 halves first (they only need the first x
        # chunk), warm bursts between units to ride out staging stalls
        for ot in (0, CT):
            emit_qk_half(ot, 0, split_evict=True)
            warm_burst(8)
        for ot in (0, CT):
            emit_qk_half(ot, 1, split_evict=True)
            warm_burst(8)
        for ot in (1, CT + 1):
            emit_qk_half(ot, 0, split_evict=True)
            warm_burst(8)
        for ot in (1, CT + 1):
            emit_qk_half(ot, 1, split_evict=True)
            warm_burst(8)

        inj_a = [lambda tt=tt, oc=oc: emit_v_half(tt, oc)
                 for tt in range(TT) for oc in range(2)]
        inj_a += [lambda ot=ot, qc=qc: emit_qk_half(ot, qc)
                  for ot in (2, CT + 2, 3, CT + 3) for qc in range(NQC)]
        inj_b = [lambda ot=ot, qc=qc: emit_qk_half(ot, qc)
                 for ot in (4, CT + 4, 5, CT + 5) for qc in range(NQC)]

        run_group((0, 1), inj_a)
        run_group((2, 3), inj_b)
        run_group((4, 5), [], last=True)

    nc.compile()
    return nc


_CACHE = {}


def _get_module():
    if "nc" not in _CACHE:
        _CACHE["nc"] = build_module()
    return _CACHE["nc"]


def _prep_in_maps(x, W_attn, b_attn, n_cores=8):
    bf = ml_dtypes.bfloat16
    OT = 2 * C // P
    Wc = np.ascontiguousarray(W_attn.astype(np.float32).T)  # [c, o]
    Wc = Wc.reshape(CT, P, 3 * C)                           # [ct, p, o]
    blkA0 = np.concatenate([Wc[:, :, 0:P], Wc[:, :, C:C + P]], axis=2)
    blkA1 = np.concatenate([Wc[:, :, P:2 * P], Wc[:, :, C + P:C + 2 * P]],
                           axis=2)
    blkB = Wc[:, :, 2 * C:3 * C]
    blkC = np.concatenate([Wc[:, :, 256:C], Wc[:, :, C + 256:2 * C]], axis=2)
    wS = np.concatenate(
        [b.transpose(1, 0, 2).reshape(P, -1)
         for b in (blkA0, blkA1, blkB, blkC)],
        axis=1)
    wS = np.ascontiguousarray(wS).astype(bf)                # [P, W_TOTAL]
    bqk = np.ascontiguousarray(
        b_attn[:2 * C].astype(np.float32).reshape(OT, P).T)  # [P, OT]
    bv = np.ascontiguousarray(
        np.tile(b_attn[2 * C:].astype(np.float32)[None, :], (P, 1))).astype(bf)
    in_maps = []
    for c in range(n_cores):
        xT_b = np.ascontiguousarray(x[c].astype(np.float32).T).astype(bf)
        in_maps.append({"xT": xT_b, "wS": wS, "bqk": bqk, "bv": bv})
    return in_maps


def run(x, W_attn, b_attn, trace=False):
    nc = _get_module()
    in_maps = _prep_in_maps(x, W_attn, b_attn)
    res = bass_utils.run_bass_kernel_spmd(
        nc, in_maps, core_ids=list(range(8)), trace=trace)
    y = np.stack([np.asarray(res.results[c]["yT"]).astype(np.float32).T
                  for c in range(8)])
    return np.ascontiguousarray(y), res


def kernel(x, W_attn, b_attn):
    y, _ = run(x, W_attn, b_attn, trace=False)
    return y

